# revision 23
# baseline (speedup 1.0000x reference)
# Depthwise causal conv1d (B=8, T=4096, C=1024, K=4, dilation=1) on 8 TRN2
# NeuronCores.
#
# Math: y[b, t, c] = sum_{j=0..3} weight[c, 3-j] * x[b, t-j, c]   (x[t<0] = 0)
#
# Strategy (v8 — fp16 I/O, PE+DVE compute split, ACT-built diag weights):
#   - Shard batch: core b handles x[b] (one full (T, C) slice).
#   - Host transposes each shard to (C, T) and casts to fp16, halving DMA
#     traffic vs fp32: ~8.4MB in + 8.4MB out per core against the ~360 GB/s
#     per-core DMA roofline (shared by loads+stores) -> ~47us floor.
#     fp16 keeps 11 significand bits: worst-case abs err ~1e-2 against an
#     output scale of ~3.2 (gate is 2e-2 relative).
#   - Per 128-channel block the 8 512-col subtiles are split across engines
#     so compute hides under DMA:
#       * 5 subtiles -> TensorE: 4 accumulating matmuls against fp16
#         diagonal weight blocks (PSUM does the tap sum); ACT copies
#         PSUM->SBUF [128,1024] at a time (two banks per copy) with an
#         inline fp32->fp16 cast.
#       * 3 subtiles -> DVE as one slab: 4 tensor_scalar mults (4x_2p mode)
#         + 3 tensor_tensor adds (2x_1p mode).
#   - Diagonal weight blocks are built on ACT (activation-copy of a DMA'd
#     identity with a per-partition scale), pipelined two blocks ahead.
#   - Ring assignment exploits startup asymmetry: the ACT HWDGE ring can
#     issue DMAs ~3us before the SP ring (SP runs the sem-init preamble),
#     so the consts and the first three x pieces go out on ACT; all other
#     loads are prefetched on SP up front (deep bufs hold a full block
#     set), and stores ride SP behind them, keeping the serialized
#     per-core DMA resource busy end to end.
#   - A few discarded matmuls ramp the PE out of its cold p-state (cold
#     ~0.9GHz / mid 1.2GHz / full 2.4GHz after ~10us of sustained work).
#   - Blocks 0 and 7 run DVE-first/PE-last: block 0 so DVE starts off the
#     first x quarter, block 7 so the final store waits only on the
#     PE->ACT chain.

import numpy as np

B, T, C, K = 8, 4096, 1024, 4
N_CORES = 8
P = 128  # SBUF partitions
NSUB = 512  # PE subtile width (one fp32 PSUM bank)
HALO = 4  # leading zero columns (causal left pad), shipped from host
PE_SUB = 5  # PE subtiles per block (of 8); the other 3 go to DVE
N_WARMUP = 3  # discarded matmuls to ramp the PE p-state during DMA latency

_CACHE = {}


def _build_nc():
    import concourse.mybir as mybir
    import concourse.tile as tile
    from concourse import bacc

    f32 = mybir.dt.float32
    f16 = mybir.dt.float16
    add = mybir.AluOpType.add
    ncb = C // P  # channel blocks per core
    half = T // 2
    hh = half + HALO

    nc = bacc.Bacc(None)
    x = nc.declare_dram_parameter("x", [C, T + HALO], f16, isOutput=False)
    # w_sb[p, cb*K + jj] = weight[cb*128 + p, jj]  (fp32 per-partition scalars)
    w = nc.declare_dram_parameter("w", [P, ncb * K], f32, isOutput=False)
    identity = nc.declare_dram_parameter("ident", [P, P], f16, isOutput=False)
    y = nc.declare_dram_parameter("y", [C, T], f16, isOutput=True)

    with tile.TileContext(nc) as tc:
        with (
            tc.tile_pool(name="const", bufs=1) as cpool,
            tc.tile_pool(name="xin", bufs=8) as xpool,
            tc.tile_pool(name="yout", bufs=8) as ypool,
            tc.tile_pool(name="tmp", bufs=2) as tpool,
            tc.tile_pool(name="ps", bufs=3, space="PSUM") as pspool,
        ):
            # PE p-state warm-up: matmuls on a just-memset scratch tile
            # (results discarded), racing the first x load's latency.
            scratch = cpool.tile([P, NSUB], f16)
            nc.gpsimd.memset(scratch[:, :], 0.0)
            for _ in range(N_WARMUP):
                psw = pspool.tile([P, NSUB], f32, tag="warm", bufs=1)
                nc.tensor.matmul(
                    psw[:, :], scratch[:, :P], scratch[:, :], start=True, stop=True
                )

            # All loads share the ACT HWDGE ring (it starts ~3us before the
            # SP ring, and a single load queue completes in order without
            # descriptor interleaving stretching the head pieces); stores
            # get the SP ring to themselves.
            w_sb = cpool.tile([P, ncb * K], f32)
            nc.scalar.dma_start(out=w_sb[:, :], in_=w[:, :])
            ident = cpool.tile([P, P], f16)
            nc.scalar.dma_start(out=ident[:, :], in_=identity[:, :])

            q = half // 2
            x_parts = [None] * ncb  # per block: list of (lo, hi, tile)

            def load_x(cb):
                rows = slice(cb * P, (cb + 1) * P)
                if cb == 0:
                    xa0 = xpool.tile([P, q + HALO], f16, tag="xa0", bufs=1)
                    xa1 = xpool.tile([P, q + HALO], f16, tag="xa1", bufs=1)
                    nc.scalar.dma_start(out=xa0[:, :], in_=x[rows, : q + HALO])
                    nc.scalar.dma_start(out=xa1[:, :], in_=x[rows, q:hh])
                    parts = [(0, q + HALO, xa0), (q, hh, xa1)]
                else:
                    xta = xpool.tile([P, hh], f16, tag="xta")
                    nc.scalar.dma_start(out=xta[:, :], in_=x[rows, :hh])
                    parts = [(0, hh, xta)]
                xtb = xpool.tile([P, hh], f16, tag="xtb")
                nc.scalar.dma_start(out=xtb[:, :], in_=x[rows, half : T + HALO])
                parts.append((half, T + HALO, xtb))
                x_parts[cb] = parts

            def wcol_of(cb, j):
                col = cb * K + (K - 1 - j)
                return w_sb[:, col : col + 1]

            # wdiag[(cb, j)] = diag(weight[cb*128+p, K-1-j]) fp16, built on
            # ACT: activation-copy of the identity, scaled per partition.
            wdiag = {}

            def build_wdiag(cb):
                for j in range(K):
                    wd = cpool.tile([P, P], f16, tag=f"wd_{cb}_{j}", name="wd")
                    nc.scalar.mul(wd[:, :], ident[:, :], wcol_of(cb, j))
                    wdiag[(cb, j)] = wd

            load_x(0)
            build_wdiag(0)
            if ncb > 1:
                load_x(1)
                build_wdiag(1)

            for cb in range(ncb):
                flip = cb == 0 or cb == ncb - 1
                rows = slice(cb * P, (cb + 1) * P)
                if cb + 2 < ncb:
                    load_x(cb + 2)
                    build_wdiag(cb + 2)

                def x_ap(lo, hi):  # global x cols [lo, hi)
                    for plo, phi, t in x_parts[cb]:
                        if lo >= plo and hi <= phi:
                            return t[:, lo - plo : hi - plo]
                    raise AssertionError((lo, hi))

                yt0 = ypool.tile([P, half], f16, tag="yt0")
                yt1 = ypool.tile([P, half], f16, tag="yt1")

                def y_ap(lo, hi):  # global y cols [lo, hi)
                    if hi <= half:
                        return yt0[:, lo:hi]
                    assert lo >= half
                    return yt1[:, lo - half : hi - half]

                # Subtile layout: normally PE takes 0..4 and DVE 5..7;
                # flipped blocks use DVE 0..2, PE 3..7.  The DVE slab is
                # emitted first so the stores issued inside the PE section
                # pick up its tile deps.
                if flip:
                    # block 0's slab must not span the xa0/xa1 seam
                    slabs = [(0, 1024), (1024, 512)] if cb == 0 else [(0, 1536)]
                    # (subtiles, y-span, store-after): pairs use 2 PSUM banks
                    pe_groups = [
                        ((3,), 0),
                        ((4, 5), None),
                        ((6, 7), 1),
                    ]
                else:
                    slabs = [(PE_SUB * NSUB, (8 - PE_SUB) * NSUB)]
                    pe_groups = [
                        ((0, 1), None),
                        ((2, 3), 0),
                        ((4,), 1),
                    ]

                # --- DVE slab: y[:, s:s+L] = sum_j w_j * x[:, s-j:s-j+L] ---
                for s, L in slabs:

                    def xoff(j):
                        off = HALO + s - j
                        return x_ap(off, off + L)

                    a = tpool.tile([P, L], f16, tag="a")
                    bb = tpool.tile([P, L], f16, tag="b")
                    cc = tpool.tile([P, L], f16, tag="c")
                    dd = tpool.tile([P, L], f16, tag="d")
                    wc = [wcol_of(cb, j) for j in range(K)]
                    nc.vector.tensor_scalar_mul(out=a[:, :], in0=xoff(0), scalar1=wc[0])
                    nc.vector.tensor_scalar_mul(
                        out=bb[:, :], in0=xoff(1), scalar1=wc[1]
                    )
                    nc.vector.tensor_tensor(
                        out=a[:, :], in0=a[:, :], in1=bb[:, :], op=add
                    )
                    nc.vector.tensor_scalar_mul(
                        out=cc[:, :], in0=xoff(2), scalar1=wc[2]
                    )
                    nc.vector.tensor_scalar_mul(
                        out=dd[:, :], in0=xoff(3), scalar1=wc[3]
                    )
                    nc.vector.tensor_tensor(
                        out=cc[:, :], in0=cc[:, :], in1=dd[:, :], op=add
                    )
                    nc.vector.tensor_tensor(
                        out=y_ap(s, s + L), in0=a[:, :], in1=cc[:, :], op=add
                    )

                # --- PE groups (1-2 subtiles per PSUM tile, one ACT copy) ---
                for ms, store_half in pe_groups:
                    n = len(ms)
                    tag = "pp" if n == 2 else "psg"
                    ps = pspool.tile([P, n * NSUB], f32, tag=tag, bufs=3 if n == 2 else 1)
                    for i, m in enumerate(ms):
                        for j in range(K):
                            off = HALO + NSUB * m - j
                            nc.tensor.matmul(
                                ps[:, i * NSUB : (i + 1) * NSUB],
                                wdiag[(cb, j)][:, :],
                                x_ap(off, off + NSUB),
                                start=(j == 0),
                                stop=(j == K - 1),
                            )
                    lo = NSUB * ms[0]
                    nc.scalar.copy(y_ap(lo, lo + n * NSUB), ps[:, :])
                    if store_half == 0:
                        nc.sync.dma_start(out=y[rows, :half], in_=yt0[:, :])
                    elif store_half == 1:
                        nc.sync.dma_start(out=y[rows, half:], in_=yt1[:, :])
    return nc


def _get_nc():
    if "nc" not in _CACHE:
        nc = _build_nc()
        nc.finalize()
        _CACHE["nc"] = nc
    return _CACHE["nc"]


def _pack_weight(weight):
    # w_sb[p, cb*K + jj] = weight[cb*P + p, jj]
    w = np.asarray(weight, dtype=np.float32)
    ncb = C // P
    return np.ascontiguousarray(
        w.reshape(ncb, P, K).transpose(1, 0, 2).reshape(P, ncb * K)
    )


def _prep_inputs(x, weight):
    x = np.asarray(x)
    w_sb = _pack_weight(weight)
    ident = np.eye(P, dtype=np.float16)
    in_maps = []
    for b in range(N_CORES):
        xt = np.zeros((C, T + HALO), dtype=np.float16)
        xt[:, HALO:] = x[b].T
        in_maps.append({"x": xt, "w": w_sb, "ident": ident})
    return in_maps


def _collect_output(res):
    y = np.empty((B, T, C), dtype=np.float32)
    for b in range(N_CORES):
        y[b] = res.results[b]["y"].T.astype(np.float32)
    return y


LAST_RESULT = None


def kernel(x, weight):
    global LAST_RESULT
    from concourse.bass_utils import run_bass_kernel_spmd

    in_maps = _prep_inputs(x, weight)
    nc = _get_nc()
    res = run_bass_kernel_spmd(nc, in_maps, list(range(N_CORES)))
    LAST_RESULT = res
    return _collect_output(res)
